# revision 1
# baseline (speedup 1.0000x reference)
"""DistMult decoder edge-scoring kernel for Trainium2 (8 NeuronCores).

score[e] = sum_d z[src_e, d] * rel_emb[type_e, d] * z[dst_e, d]

Sharding: pure edge-parallel across 8 cores; z and rel_emb replicated.

Edges per core are bucketed by (src//25000, dst//25000) into 16 buckets so
z-row indices fit int16 against one of four z quarter-tables. Each bucket is
padded to CAP slots; slot i of a bucket lands at [i%128, i//128] in the
bucket's gathered tile (dma_gather layout). Gathers round-robin over 4 SWDGE
queues with one DMA semaphore per queue; the vector engine runs
mult/mult/reduce per bucket with ping-pong buffers. Host un-permutes scores.

Buckets are padded with -1 indices; the gather firmware trims trailing
negatives (count supplied per bucket via a gpsimd register), so padding costs
no HBM traffic. rel_emb is replicated 256x in DRAM to spread HBM row
conflicts of the hot 100-row table. If a bucket ever exceeds CAP slots the
kernel transparently recompiles with a larger capacity (safe up to ~6400
slots/bucket, far beyond anything uniform edge distributions produce).

Measured on 8 axon trn2 cores: ~0.7-1.0 ms end-to-end per call (machine-state
dependent), vs 2.8 ms for the naive per-column indirect-DMA version.
"""

import numpy as np

import concourse.bass as bass
from concourse import bacc, mybir
from concourse.bass_utils import run_bass_kernel_spmd

N_NODES = 100000
N_REL = 100
HIDDEN = 128
N_EDGES = 600000
N_CORES = 8
E_CORE = N_EDGES // N_CORES   # 75000
P = 128
NQ = 4                        # z quarter tables
ZQ = N_NODES // NQ            # 25000 rows per quarter
NB = NQ * NQ                  # 16 buckets
CAP = 5632                    # slots per bucket (mean 4687 + 14 sigma)
NSETS = 2                     # ping-pong buffer sets
RELREP = 256                  # rel_emb DRAM replication (spreads HBM row conflicts)

_cache = {}


def _build(cap=CAP, reps=1, compute=True, nsets=NSETS):
    """reps>1 repeats the full bucket sweep (for wall-clock amplification).
    compute=False drops the vector stage (pure gather throughput bench)."""
    cols = cap // P
    ci = cap // 16
    f32, i16 = mybir.dt.float32, mybir.dt.int16
    nc = bacc.Bacc("TRN2", target_bir_lowering=False, debug=False,
                   num_swdge_queues=4)

    zt = [nc.dram_tensor(f"z{q}", [ZQ, HIDDEN], f32,
                         kind="ExternalInput").ap() for q in range(NQ)]
    rel = nc.dram_tensor("rel", [RELREP * N_REL, HIDDEN], f32,
                         kind="ExternalInput").ap()
    sidx = nc.dram_tensor("sidx", [P, NB * ci], i16, kind="ExternalInput").ap()
    didx = nc.dram_tensor("didx", [P, NB * ci], i16, kind="ExternalInput").ap()
    tidx = nc.dram_tensor("tidx", [P, NB * ci], i16, kind="ExternalInput").ap()
    bcnt = nc.dram_tensor("bcnt", [1, NB], mybir.dt.int32,
                          kind="ExternalInput").ap()
    out = nc.dram_tensor("out", [P, NB * cols], f32, kind="ExternalOutput").ap()

    from contextlib import ExitStack
    with (
        nc.Block() as block,
        nc.sbuf_tensor("sidx_sb", [P, NB * ci], i16) as sidx_sb,
        nc.sbuf_tensor("didx_sb", [P, NB * ci], i16) as didx_sb,
        nc.sbuf_tensor("tidx_sb", [P, NB * ci], i16) as tidx_sb,
        nc.sbuf_tensor("bcnt_sb", [1, NB], mybir.dt.int32) as bcnt_sb,
        nc.sbuf_tensor("scores", [P, NB * cols], f32) as scores,
        nc.semaphore("io") as io,
        nc.semaphore("vdone") as vdone,
        nc.semaphore("vaux") as vaux,
        ExitStack() as stack,
    ):
        qsem = [[stack.enter_context(nc.semaphore(f"q{j}s{s}"))  # noqa: ANT232
                 for s in range(nsets)] for j in range(4)]
        gbuf = []
        for s in range(nsets):
            bufs = []
            for nm in ("src", "dst", "rel"):
                bufs.append(stack.enter_context(
                    nc.sbuf_tensor(f"{nm}g{s}", [P, cols, HIDDEN], f32)))
            gbuf.append(bufs)

        total = reps * NB

        @block.sync
        def _(sync: bass.BassEngine):
            sync.dma_start(out=sidx_sb[:], in_=sidx[:]).then_inc(io, 16)
            sync.dma_start(out=didx_sb[:], in_=didx[:]).then_inc(io, 16)
            sync.dma_start(out=tidx_sb[:], in_=tidx[:]).then_inc(io, 16)
            sync.dma_start(out=bcnt_sb[:], in_=bcnt[:]).then_inc(io, 16)
            if compute:
                sync.wait_ge(vdone, total)
            else:
                gtot = 3 * total
                for j in range(4):
                    for s_ in range(nsets):
                        n = sum(1 for g in range(gtot)
                                if g % 4 == j and (g // 3) % nsets == s_)
                        if n:
                            sync.wait_ge(qsem[j][s_], 16 * n)
            sync.dma_start(out=out[:], in_=scores[:]).then_inc(io, 16)
            sync.wait_ge(io, 64)

        @block.gpsimd
        def _(gp: bass.BassGpSimd):
            gp.wait_ge(io, 64)
            g = 0
            creg_cm = gp.register("bcnt_reg")
            creg = creg_cm.__enter__()
            for it in range(total):
                b = it % NB
                if compute and it >= nsets:
                    gp.wait_ge(vdone, it - nsets + 1)
                s_ = it % nsets
                st = gbuf[s_]
                qs, qd = b // NQ, b % NQ
                gp.reg_load(creg, bcnt_sb[0:1, b:b + 1])
                for buf, tab, isb in ((st[0], zt[qs], sidx_sb),
                                      (st[1], zt[qd], didx_sb),
                                      (st[2], rel, tidx_sb)):
                    q = g % 4
                    gp.dma_gather(
                        buf[:], tab[:], isb[:, b * ci:(b + 1) * ci],
                        cap, creg, HIDDEN,
                        single_packet=False, queue_num=q,
                    ).then_inc(qsem[q][s_], 16)
                    g += 1
            creg_cm.__exit__(None, None, None)

        @block.vector
        def _(v: bass.BassVectorEngine):
            if not compute:
                return
            cnt = [[0] * nsets for _ in range(4)]
            g = 0
            for it in range(total):
                b = it % NB
                s_ = it % nsets
                st = gbuf[s_]
                changed = set()
                for _s in range(3):
                    cnt[g % 4][s_] += 1
                    changed.add(g % 4)
                    g += 1
                for j in sorted(changed):
                    v.wait_ge(qsem[j][s_], 16 * cnt[j][s_])
                v.tensor_tensor(out=st[0][:], in0=st[0][:], in1=st[1][:],
                                op=mybir.AluOpType.mult).then_inc(vaux, 1)
                v.tensor_tensor(out=st[0][:], in0=st[0][:], in1=st[2][:],
                                op=mybir.AluOpType.mult,
                                )._wait_ge(vaux, 2 * it + 1).then_inc(vaux, 1)
                v.tensor_reduce(
                    out=scores[:, b * cols:(b + 1) * cols], in_=st[0][:],
                    axis=mybir.AxisListType.X, op=mybir.AluOpType.add,
                )._wait_ge(vaux, 2 * it + 2).then_inc(vdone, 1)

    nc.compile()
    return nc


def _wrap(idx2d):
    """[NB, CAP] int -> wrapped [128, NB*CI] int16."""
    nb, cap = idx2d.shape
    w = idx2d.reshape(nb, cap // 16, 16).transpose(0, 2, 1)  # [NB,16,CI]
    w = np.tile(w, (1, 8, 1))                                # [NB,128,CI]
    return np.concatenate(list(w), axis=1).astype(np.int16)  # [128, NB*CI]


def _prep_inputs(z, rel_emb, edge_index, edge_type, cap=CAP):
    cols = cap // P
    z = np.ascontiguousarray(z, dtype=np.float32)
    rel_emb = np.ascontiguousarray(rel_emb, dtype=np.float32)
    src = np.asarray(edge_index[0], dtype=np.int64)
    dst = np.asarray(edge_index[1], dtype=np.int64)
    typ = np.asarray(edge_type, dtype=np.int64)

    zq = [np.ascontiguousarray(z[q * ZQ:(q + 1) * ZQ]) for q in range(NQ)]
    rel_rep = np.ascontiguousarray(np.tile(rel_emb, (RELREP, 1)))

    in_maps, positions = [], []
    for c in range(N_CORES):
        sl = slice(c * E_CORE, (c + 1) * E_CORE)
        s, d, t = src[sl], dst[sl], typ[sl]
        b = (s // ZQ) * NQ + (d // ZQ)
        order = np.argsort(b, kind="stable")
        counts = np.bincount(b, minlength=NB)
        if counts.max() > cap:
            raise OverflowError(int(counts.max()))
        starts = np.zeros(NB, np.int64)
        starts[1:] = np.cumsum(counts)[:-1]
        rank = np.arange(E_CORE) - starts[b[order]]
        bo = b[order]

        sloc = np.full((NB, cap), -1, np.int64)
        dloc = np.full((NB, cap), -1, np.int64)
        tloc = np.full((NB, cap), -1, np.int64)
        sloc[bo, rank] = s[order] % ZQ
        dloc[bo, rank] = d[order] % ZQ
        tloc[bo, rank] = t[order] + N_REL * (rank % RELREP)

        # score of (bucket bb, slot r) lands at out[r%128, bb*cols + r//128]
        pos = np.empty(E_CORE, np.int64)
        pos[order] = (rank % P) * (NB * cols) + bo * cols + rank // P
        positions.append(pos)

        cnts = np.maximum(counts, 1).astype(np.int32)
        for bb in range(NB):
            if counts[bb] == 0:
                sloc[bb, 0] = 0; dloc[bb, 0] = 0; tloc[bb, 0] = 0
        in_maps.append({
            **{f"z{q}": zq[q] for q in range(NQ)},
            "rel": rel_rep,
            "bcnt": cnts.reshape(1, NB),
            "sidx": _wrap(sloc),
            "didx": _wrap(dloc),
            "tidx": _wrap(tloc),
        })
    return in_maps, positions


def kernel_run(z, rel_emb, edge_index, edge_type, trace=False):
    cap = _cache.get("cap", CAP)
    while True:
        try:
            in_maps, positions = _prep_inputs(z, rel_emb, edge_index,
                                              edge_type, cap=cap)
            break
        except OverflowError as e:
            cap = -(-int(e.args[0]) // P) * P
            _cache.pop("nc", None)
            _cache["cap"] = cap
    if "nc" not in _cache:
        _cache["nc"] = _build(cap=cap)
    nc = _cache["nc"]
    res = run_bass_kernel_spmd(nc, in_maps, core_ids=list(range(N_CORES)),
                               trace=trace)
    parts = [np.asarray(res.results[c]["out"]).reshape(-1)[positions[c]]
             for c in range(N_CORES)]
    return np.concatenate(parts).astype(np.float32, copy=False), res


def kernel(z, rel_emb, edge_index, edge_type):
    out, _ = kernel_run(z, rel_emb, edge_index, edge_type)
    return out



# revision 2
# speedup vs baseline: 1.1983x; 1.1983x over previous
"""DistMult decoder edge-scoring kernel for Trainium2 (8 NeuronCores).

score[e] = sum_d z[src_e, d] * rel_emb[type_e, d] * z[dst_e, d]

Sharding: pure edge-parallel across 8 cores; z and rel_emb replicated.

Per core: edges are bucketed by (src//25000, dst//25000) into 16 buckets so
z-row indices fit int16 against one of four fp16 z quarter tables (dma_gather
indices must be int16; fp16 halves HBM gather bytes, rel err ~7e-4 vs the
2e-2 gate). Within a bucket, edges of one relation type are packed into
partition rows: bulk regions are 8 columns wide holding only full 8-edge
single-type rows (zero pad); remainder edges go to 1-column spill regions
(one edge per row, trailing unused rows trimmed via -1 indices + per-bucket
count registers, so padding costs no descriptors). The rel factor is applied
per region with one tensor_tensor whose in1 is a [128,1,128] slice of a
per-(region,partition) rel table broadcast (stride 0) across the region's
columns — eliminating the per-edge rel gather (1/3 of all DMA descriptors)
entirely. Exactly 150k descriptors/core (2 per edge), the minimum for this
gather design.

32 per-bucket gathers (src+dst interleaved) round-robin over 4 SWDGE queues
(the hard per-queue descriptor rate ~7-9 ns/desc is the bottleneck; finer
granularity than per-pair gathers measurably improves queue overlap), 3
ping-pong buffer sets so DVE compute (mult, per-region rel-mult, reduce)
fully hides under the gathers. Host un-permutes scores.

Measured on 8 axon trn2 cores: ~0.28-0.30 ms steady-state sweep (reps-slope)
vs 0.74-1.0 ms for the previous f32 3-gather version.
"""

import numpy as np

import concourse.bass as bass
from concourse import bacc, mybir

N_NODES = 100000
N_REL = 100
HIDDEN = 128
N_EDGES = 600000
N_CORES = 8
E_CORE = N_EDGES // N_CORES
P = 128
NQ = 4
ZQ = N_NODES // NQ
NB = NQ * NQ            # 16 buckets
W = 8                   # bulk region width
NSETS = 3

_cache = {}


def _group_core(s, d, t):
    """Per bucket: bulk rows (type, 8 edge ids) and spill (type, edge id)."""
    b = (s // ZQ) * NQ + (d // ZQ)
    key = b * N_REL + t
    order = np.argsort(key, kind="stable")
    ks = key[order]
    grp_ids, grp_starts, grp_counts = np.unique(
        ks, return_index=True, return_counts=True)
    bulk = [[] for _ in range(NB)]    # [(type, [edge ids x8])]
    spill = [[] for _ in range(NB)]   # [(type, edge id)]
    for gi in range(len(grp_ids)):
        bb = int(grp_ids[gi]) // N_REL
        tt = int(grp_ids[gi]) % N_REL
        e = order[grp_starts[gi]:grp_starts[gi] + grp_counts[gi]]
        nfull = len(e) // W
        for k in range(nfull):
            bulk[bb].append((tt, e[k * W:(k + 1) * W]))
        for x in e[nfull * W:]:
            spill[bb].append((tt, x))
    return bulk, spill


def _choose_shapes(all_bulk, all_spill):
    """Static (NB8, NS) per bucket minimizing capacity, feasible for all
    cores (surplus bulk rows demote to spill rows, 8 spill rows each)."""
    shapes = []
    for bb in range(NB):
        R8 = [len(all_bulk[c][bb]) for c in range(N_CORES)]
        rem = [len(all_spill[c][bb]) for c in range(N_CORES)]
        best = None
        for nb8 in range(0, max(R8) // P + 2):
            need = [rem[c] + W * max(0, R8[c] - P * nb8)
                    for c in range(N_CORES)]
            ns = max(-(-n // P) for n in need) if max(need) > 0 else 0
            cap = P * (W * nb8 + ns)
            # 2 gather descriptors per slot; ~70ns-equivalent penalty per
            # region (DVE instruction + relt SBUF footprint)
            cost = 2 * cap + 70 * (nb8 + ns)
            if best is None or cost < best[0]:
                best = (cost, nb8, ns)
        shapes.append((best[1], best[2]))
    return tuple(shapes)


def _wrap(flat_idx):
    """[S] int -> wrapped [128, S//16] int16 (idx i at [i%16, i//16], x8)."""
    S = flat_idx.shape[0]
    w = flat_idx.reshape(S // 16, 16).T
    return np.tile(w, (8, 1)).astype(np.int16)


def _shape_geom(shape_key):
    nb8 = np.array([x[0] for x in shape_key])
    ns = np.array([x[1] for x in shape_key])
    widths = [[W] * int(nb8[b]) + [1] * int(ns[b]) for b in range(NB)]
    cols = np.array([sum(w) for w in widths], np.int64)
    nreg = np.array([len(w) for w in widths], np.int64)
    colbase = np.concatenate([[0], np.cumsum(cols)])[:-1]
    regbase = np.concatenate([[0], np.cumsum(nreg)])[:-1]
    return widths, cols, nreg, colbase, regbase


def _prep_inputs(z, rel_emb, edge_index, edge_type):
    z = np.asarray(z, dtype=np.float32).astype(np.float16)
    rel_emb = np.asarray(rel_emb, dtype=np.float32).astype(np.float16)
    src = np.asarray(edge_index[0], dtype=np.int64)
    dst = np.asarray(edge_index[1], dtype=np.int64)
    typ = np.asarray(edge_type, dtype=np.int64)

    zq = [np.ascontiguousarray(z[q * ZQ:(q + 1) * ZQ]) for q in range(NQ)]

    all_bulk, all_spill = [], []
    for c in range(N_CORES):
        sl = slice(c * E_CORE, (c + 1) * E_CORE)
        bulk, spill = _group_core(src[sl], dst[sl], typ[sl])
        all_bulk.append(bulk)
        all_spill.append(spill)

    shape_key = _choose_shapes(all_bulk, all_spill)
    widths, cols, nreg, colbase, regbase = _shape_geom(shape_key)
    total_cols = int(cols.sum())
    gtot = int(nreg.sum())

    in_maps, positions = [], []
    for c in range(N_CORES):
        sl = slice(c * E_CORE, (c + 1) * E_CORE)
        s_, d_, t_ = src[sl], dst[sl], typ[sl]
        sflat = np.zeros(total_cols * P, np.int64)
        dflat = np.zeros(total_cols * P, np.int64)
        pos = np.empty(E_CORE, np.int64)
        tor = np.zeros((gtot, P), np.int64)
        vcnt = np.zeros(NB, np.int32)

        for bb in range(NB):
            nb8, ns = shape_key[bb]
            bulk = all_bulk[c][bb]
            spill = list(all_spill[c][bb])
            if len(bulk) > P * nb8:
                for tt, e8 in bulk[P * nb8:]:
                    spill.extend((tt, x) for x in e8)
                bulk = bulk[:P * nb8]
            assert len(spill) <= P * ns, (bb, len(spill), ns)

            for i, (tt, e8) in enumerate(bulk):
                rg, p = i // P, i % P
                tor[regbase[bb] + rg, p] = tt
                c0 = colbase[bb] + rg * W
                for k, e in enumerate(e8):
                    sflat[(c0 + k) * P + p] = s_[e] % ZQ
                    dflat[(c0 + k) * P + p] = d_[e] % ZQ
                    pos[e] = p * total_cols + c0 + k
            for j, (tt, e) in enumerate(spill):
                rg, p = j // P, j % P
                tor[regbase[bb] + nb8 + rg, p] = tt
                cc = colbase[bb] + nb8 * W + rg
                sflat[cc * P + p] = s_[e] % ZQ
                dflat[cc * P + p] = d_[e] % ZQ
                pos[e] = p * total_cols + cc
            # trailing spill rows are unused: mark -1 so the gather
            # firmware skips them (count reg = valid prefix length)
            base = (colbase[bb] + nb8 * W) * P
            sflat[base + len(spill):base + P * ns] = -1
            dflat[base + len(spill):base + P * ns] = -1
            vcnt[bb] = int(cols[bb]) * P - (P * ns - len(spill))

        relt = rel_emb[tor]                    # [gtot, 128, 128]
        relt = np.ascontiguousarray(
            relt.transpose(1, 0, 2).reshape(P, gtot * HIDDEN))
        positions.append(pos)
        in_maps.append({
            **{f"z{q}": zq[q] for q in range(NQ)},
            "relt": relt,
            "bcnt": vcnt.reshape(1, NB),
            "sidx": _wrap(sflat),
            "didx": _wrap(dflat),
        })
    return in_maps, positions, shape_key


def _build(shape_key, reps=1, nsets=NSETS, compute=True,
           src_split=True, scratch=16384, chunks=1):
    widths, cols, nreg, colbase, regbase = _shape_geom(shape_key)
    total_cols = int(cols.sum())
    gtot = int(nreg.sum())
    total_slots = total_cols * P
    pcols = [int(cols[2 * k] + cols[2 * k + 1]) for k in range(NB // 2)]
    maxpc = max(pcols)

    f16, f32, i16 = mybir.dt.float16, mybir.dt.float32, mybir.dt.int16
    nc = bacc.Bacc("TRN2", target_bir_lowering=False, debug=False,
                   num_swdge_queues=4, dynamic_dma_scratch_size=scratch)

    zt = [nc.dram_tensor(f"z{q}", [ZQ, HIDDEN], f16,
                         kind="ExternalInput").ap() for q in range(NQ)]
    relt_d = nc.dram_tensor("relt", [P, gtot * HIDDEN], f16,
                            kind="ExternalInput").ap()
    sidx = nc.dram_tensor("sidx", [P, total_slots // 16], i16,
                          kind="ExternalInput").ap()
    didx = nc.dram_tensor("didx", [P, total_slots // 16], i16,
                          kind="ExternalInput").ap()
    bcnt = nc.dram_tensor("bcnt", [1, NB], mybir.dt.int32,
                          kind="ExternalInput").ap()
    out = nc.dram_tensor("out", [P, total_cols], f32,
                         kind="ExternalOutput").ap()

    from contextlib import ExitStack
    with (
        nc.Block() as block,
        nc.sbuf_tensor("sidx_sb", [P, total_slots // 16], i16) as sidx_sb,
        nc.sbuf_tensor("didx_sb", [P, total_slots // 16], i16) as didx_sb,
        nc.sbuf_tensor("relt_sb", [P, gtot * HIDDEN], f16) as relt_sb,
        nc.sbuf_tensor("bcnt_sb", [1, NB], mybir.dt.int32) as bcnt_sb,
        nc.sbuf_tensor("scores", [P, total_cols], f32) as scores,
        nc.semaphore("io") as io,
        nc.semaphore("vdone") as vdone,
        nc.semaphore("vaux") as vaux,
        ExitStack() as stack,
    ):
        qsem = [[stack.enter_context(nc.semaphore(f"q{j}s{s}"))  # noqa: ANT232
                 for s in range(nsets)] for j in range(4)]
        gsrc = [stack.enter_context(
            nc.sbuf_tensor(f"gs{s}", [P, maxpc, HIDDEN], f16))
            for s in range(nsets)]
        gdst = [stack.enter_context(
            nc.sbuf_tensor(f"gd{s}", [P, maxpc, HIDDEN], f16))
            for s in range(nsets)]

        npairs = NB // 2
        total = reps * npairs

        def _chunked(plan):
            out = []
            for kind, boff, tab, slot0, nsl in plan:
                ncols = nsl // P
                splits = [ncols // chunks + (1 if i < ncols % chunks else 0)
                          for i in range(chunks)]
                off = 0
                for sp in splits:
                    if sp:
                        out.append((kind, boff + off, tab,
                                    slot0 + off * P, sp * P))
                    off += sp
            return out

        def pair_plan(pr):
            b0, b1 = 2 * pr, 2 * pr + 1
            qs = b0 // NQ
            if src_split:
                plan = [
                    ("s", 0, qs, int(colbase[b0]) * P, int(cols[b0]) * P, b0),
                    ("d", 0, b0 % NQ, int(colbase[b0]) * P,
                     int(cols[b0]) * P, b0),
                    ("s", int(cols[b0]), qs, int(colbase[b1]) * P,
                     int(cols[b1]) * P, b1),
                    ("d", int(cols[b0]), b1 % NQ, int(colbase[b1]) * P,
                     int(cols[b1]) * P, b1),
                ]
                return plan
            return [
                ("s", 0, qs, int(colbase[b0]) * P,
                 (int(cols[b0]) + int(cols[b1])) * P),
                ("d", 0, b0 % NQ, int(colbase[b0]) * P, int(cols[b0]) * P),
                ("d", int(cols[b0]), b1 % NQ, int(colbase[b1]) * P,
                 int(cols[b1]) * P),
            ]

        @block.sync
        def _(sync: bass.BassEngine):
            sync.dma_start(out=sidx_sb[:], in_=sidx[:]).then_inc(io, 16)
            sync.dma_start(out=didx_sb[:], in_=didx[:]).then_inc(io, 16)
            sync.dma_start(out=relt_sb[:], in_=relt_d[:]).then_inc(io, 16)
            sync.dma_start(out=bcnt_sb[:], in_=bcnt[:]).then_inc(io, 16)
            if compute:
                sync.wait_ge(vdone, total)
            else:
                npg = len(pair_plan(0))
                for j in range(4):
                    for s_ in range(nsets):
                        n = sum(1 for g in range(npg * total)
                                if g % 4 == j and (g // npg) % nsets == s_)
                        if n:
                            sync.wait_ge(qsem[j][s_], 16 * n)
            sync.dma_start(out=out[:], in_=scores[:]).then_inc(io, 16)
            sync.wait_ge(io, 80)

        @block.gpsimd
        def _(gp: bass.BassGpSimd):
            gp.wait_ge(io, 64)
            g = 0
            creg_cm = gp.register("bcnt_reg")
            creg = creg_cm.__enter__()
            for it in range(total):
                pr = it % npairs
                if compute and it >= nsets:
                    gp.wait_ge(vdone, it - nsets + 1)
                s_ = it % nsets
                last_b = None
                for entry in pair_plan(pr):
                    kind, boff, q_or_b, slot0, nsl = entry[:5]
                    trim_b = entry[5] if len(entry) > 5 else None
                    buf = gsrc[s_] if kind == "s" else gdst[s_]
                    isb = sidx_sb if kind == "s" else didx_sb
                    q = g % 4
                    if trim_b is not None and trim_b != last_b:
                        gp.reg_load(creg, bcnt_sb[0:1, trim_b:trim_b + 1])
                        last_b = trim_b
                    cnt_arg = creg if trim_b is not None else nsl
                    gp.dma_gather(
                        buf[:, boff:boff + nsl // P, :], zt[q_or_b][:],
                        isb[:, slot0 // 16:(slot0 + nsl) // 16],
                        nsl, cnt_arg, HIDDEN,
                        single_packet=False, queue_num=q,
                    ).then_inc(qsem[q][s_], 16)
                    g += 1
            creg_cm.__exit__(None, None, None)

        @block.vector
        def _(v: bass.BassVectorEngine):
            if not compute:
                return
            cnt = [[0] * nsets for _ in range(4)]
            g = 0
            va = 0
            for it in range(total):
                pr = it % npairs
                b0, b1 = 2 * pr, 2 * pr + 1
                s_ = it % nsets
                pc = int(cols[b0]) + int(cols[b1])
                changed = set()
                for _s in range(len(pair_plan(pr))):
                    cnt[g % 4][s_] += 1
                    changed.add(g % 4)
                    g += 1
                for j in sorted(changed):
                    v.wait_ge(qsem[j][s_], 16 * cnt[j][s_])
                src_t = gsrc[s_][:, 0:pc, :]
                dst_t = gdst[s_][:, 0:pc, :]
                v.tensor_tensor(out=src_t, in0=src_t, in1=dst_t,
                                op=mybir.AluOpType.mult).then_inc(vaux, 1)
                va += 1
                off = 0
                for bb in (b0, b1):
                    for r, w_r in enumerate(widths[bb]):
                        gidx = int(regbase[bb]) + r
                        rtile = relt_sb[:, gidx * HIDDEN:(gidx + 1) * HIDDEN]
                        rb = rtile.unsqueeze(1).broadcast_to((P, w_r, HIDDEN))
                        seg = gsrc[s_][:, off:off + w_r, :]
                        v.tensor_tensor(out=seg, in0=seg, in1=rb,
                                        op=mybir.AluOpType.mult,
                                        )._wait_ge(vaux, va).then_inc(vaux, 1)
                        va += 1
                        off += w_r
                v.tensor_reduce(
                    out=scores[:, int(colbase[b0]):int(colbase[b0]) + pc],
                    in_=src_t, axis=mybir.AxisListType.X,
                    op=mybir.AluOpType.add,
                )._wait_ge(vaux, va).then_inc(vdone, 1)

    nc.compile()
    return nc


def kernel_run(z, rel_emb, edge_index, edge_type, trace=False):
    from concourse.bass_utils import run_bass_kernel_spmd
    in_maps, positions, shape_key = _prep_inputs(z, rel_emb, edge_index,
                                                 edge_type)
    if _cache.get("key") != shape_key:
        _cache["nc"] = _build(shape_key)
        _cache["key"] = shape_key
    nc = _cache["nc"]
    res = run_bass_kernel_spmd(nc, in_maps, core_ids=list(range(N_CORES)),
                               trace=trace)
    parts = [np.asarray(res.results[c]["out"]).reshape(-1)[positions[c]]
             for c in range(N_CORES)]
    return np.concatenate(parts).astype(np.float32, copy=False), res


def kernel(z, rel_emb, edge_index, edge_type):
    out, _ = kernel_run(z, rel_emb, edge_index, edge_type)
    return out
